# revision 12
# baseline (speedup 1.0000x reference)
"""Trainium2 Bass kernel for CrossAttentionModule.

Strategy: data-parallel over batch (8 elements per core on 8 cores).
Per core, per batch element:
  x_cls_p = x_cls @ W_proj + b_proj            (computed transposed: [768, 8])
  x       = [x_cls_p; x_img]                   (577 tokens, fed transposed [768, 577])
  kT      = W_k.T-contract GEMM -> [768, 577]  (channels on partitions)
  v       = x @ W_v            -> [577, 768]   (tokens on partitions)
  scores  = blockdiag(q) @ kT  -> [12, 577], softmax over tokens
  ctx     = attn @ v via PE transposes of attn, block-diag extraction
  y       = (x_cls_p + ctx) @ W_back + b_back

All large GEMMs run as float32r (1 cycle/row on the PE at N>=256).
Host side only re-lays-out data (transposes/reshapes); all FLOPs on device.
"""

import numpy as np
from contextlib import ExitStack

import concourse.bass as bass
import concourse.tile as tile
from concourse import bacc
from concourse import mybir
from concourse.bass_utils import run_bass_kernel_spmd
from concourse.masks import make_identity

B, N_IMG, C_CLS, C, H, HD = 64, 576, 384, 768, 12, 64
NCORES = 8
BL = B // NCORES           # 8 batch elements per core
NT = 1 + N_IMG             # 577 tokens
SCALE = HD ** -0.5
F32 = mybir.dt.float32
F32R = mybir.dt.float32r
CI = C // 128              # 6 input-channel tiles
CIC = C_CLS // 128         # 3 cls-channel tiles
NTP = NT + 1               # padded to even (578) for fp32r matmul ISA rules
TT = (NTP + 127) // 128    # 5 token tiles
LAST_T = NTP - (TT - 1) * 128  # 66 (row 65 = zero pad token)


def _chunks(total, step):
    out, o = [], 0
    while o < total:
        out.append((o, min(step, total - o)))
        o += step
    return out


def _emit(nc):
    dx = nc.declare_dram_parameter("xT_img", [BL, 128, CI, N_IMG], F32R, isOutput=False)
    dxc = nc.declare_dram_parameter("x_clsT", [128, CIC, BL], F32, isOutput=False)
    dwp = nc.declare_dram_parameter("W_proj", [128, CIC, C], F32, isOutput=False)
    dbp = nc.declare_dram_parameter("b_projT", [128, CI], F32, isOutput=False)
    dwq = nc.declare_dram_parameter("W_q", [128, CI, C], F32R, isOutput=False)
    dwk = nc.declare_dram_parameter("W_k", [128, CI, C], F32R, isOutput=False)
    dwv = nc.declare_dram_parameter("W_v", [128, CI, C], F32R, isOutput=False)
    dwb = nc.declare_dram_parameter("W_back", [128, CI, C_CLS], F32R, isOutput=False)
    dbb = nc.declare_dram_parameter("b_back", [BL, C_CLS], F32, isOutput=False)
    doh = nc.declare_dram_parameter("onehot", [H, BL, BL], F32R, isOutput=False)
    dbm = nc.declare_dram_parameter("blockmask", [H, C], F32, isOutput=False)
    dy = nc.declare_dram_parameter("y", [BL, C_CLS], F32, isOutput=True)

    with tile.TileContext(nc) as tc, ExitStack() as ctx:
        const = ctx.enter_context(tc.tile_pool(name="const", bufs=1))
        wpool = ctx.enter_context(tc.tile_pool(name="wpool", bufs=1))
        xpool = ctx.enter_context(tc.tile_pool(name="xpool", bufs=2))
        kpool = ctx.enter_context(tc.tile_pool(name="kpool", bufs=2))
        vpool = ctx.enter_context(tc.tile_pool(name="vpool", bufs=2))
        spool = ctx.enter_context(tc.tile_pool(name="spool", bufs=3))
        psA = ctx.enter_context(tc.tile_pool(name="psA", bufs=4, space="PSUM"))
        psB = ctx.enter_context(tc.tile_pool(name="psB", bufs=4, space="PSUM"))

        ident = const.tile([128, 128], F32)
        make_identity(nc, ident)

        def zero(ap):
            nc.vector.memset(ap.bitcast(mybir.dt.uint32), 0)

        # weights to SBUF (per-ci DMAs spread across queues)
        wp = wpool.tile([128, CIC, C], F32)
        wq = wpool.tile([128, CI, C], F32R)
        wk = wpool.tile([128, CI, C], F32R)
        wv = wpool.tile([128, CI, C], F32R)
        wb = wpool.tile([128, CI, C_CLS], F32R)
        nc.sync.dma_start(out=wp[:], in_=dwp[:])
        nc.sync.dma_start(out=wq[:], in_=dwq[:])
        nc.sync.dma_start(out=wk[:], in_=dwk[:])
        nc.sync.dma_start(out=wv[:], in_=dwv[:])
        nc.sync.dma_start(out=wb[:], in_=dwb[:])
        xc = const.tile([128, CIC, BL], F32)
        nc.sync.dma_start(out=xc[:], in_=dxc[:])
        bp = const.tile([128, CI], F32)
        nc.sync.dma_start(out=bp[:], in_=dbp[:])
        bb = const.tile([BL, C_CLS], F32)
        nc.sync.dma_start(out=bb[:], in_=dbb[:])
        oh = const.tile([H, BL, BL], F32R)
        nc.sync.dma_start(out=oh[:], in_=doh[:])
        bm = const.tile([H, C], F32)
        nc.sync.dma_start(out=bm[:], in_=dbm[:])

        # ---- prologue: x_cls_pT [128, ci, b] = (x_cls @ W_proj + b_proj).T
        xcp = const.tile([128, CI, BL], F32R)
        for co in range(CI):
            ps = psB.tile([128, BL], F32, tag="sm")
            for ci in range(CIC):
                nc.tensor.matmul(
                    ps[:],
                    wp[:, ci, co * 128:(co + 1) * 128],
                    xc[:, ci, :],
                    start=(ci == 0),
                    stop=(ci == CIC - 1),
                )
            nc.vector.tensor_scalar_add(xcp[:, co, :], ps[:], bp[:, co:co + 1])

        # ---- q = x_cls_p @ (W_q * scale)   [8, 768] then transpose
        q_sb = const.tile([BL, C], F32)
        for off, n in _chunks(C, 512):
            ps = psB.tile([BL, 512], F32, tag="sm")
            for ci in range(CI):
                nc.tensor.matmul(
                    ps[:, :n],
                    xcp[:, ci, :],
                    wq[:, ci, off:off + n],
                    start=(ci == 0),
                    stop=(ci == CI - 1),
                )
            nc.vector.tensor_copy(q_sb[:, off:off + n], ps[:, :n])
        qT = const.tile([128, CI, BL], F32)
        for ct in range(CI):
            pst = psB.tile([128, BL], F32, tag="sm")
            nc.tensor.transpose(pst[:], q_sb[:, ct * 128:(ct + 1) * 128], ident[:BL, :BL])
            nc.vector.tensor_copy(qT[:, ct, :], pst[:])
        # block-diagonal q: qb[p, ct, b, h] = qT[p, ct, b] iff head h owns partition p of tile ct
        qb = const.tile([128, CI, BL, H], F32R)
        zero(qb[:])
        for ct in range(CI):
            nc.vector.tensor_copy(qb[0:64, ct, :, 2 * ct], qT[0:64, ct, :])
            nc.vector.tensor_copy(qb[64:128, ct, :, 2 * ct + 1], qT[64:128, ct, :])

        attn_out = const.tile([BL, C], F32)
        nc.vector.memset(attn_out[:], 0.0)

        def kv_gemm(b):
            xt = xpool.tile([128, CI, NTP], F32R, tag="xt")
            nc.sync.dma_start(out=xt[:, :, 1:NT], in_=dx[b])
            nc.sync.dma_start(out=xt[:, :, 0:1], in_=xcp[:, :, b:b + 1])
            zero(xt[:, :, NT:NTP])
            kt = kpool.tile([128, CI, NTP], F32R, tag="kt")
            for co in range(CI):
                for off, n in _chunks(NTP, 512):
                    ps = psA.tile([128, 512], F32, tag="mm")
                    for ci in range(CI):
                        nc.tensor.matmul(
                            ps[:, :n],
                            wk[:, ci, co * 128:(co + 1) * 128],
                            xt[:, ci, off:off + n],
                            start=(ci == 0),
                            stop=(ci == CI - 1),
                        )
                    nc.vector.tensor_copy(kt[:, co, off:off + n], ps[:, :n])
            vt = vpool.tile([128, TT, C], F32R, tag="vt")
            for t in range(TT):
                tn = 128 if t < TT - 1 else LAST_T
                for off, n in _chunks(C, 512):
                    ps = psA.tile([128, 512], F32, tag="mm")
                    for ci in range(CI):
                        nc.tensor.matmul(
                            ps[:tn, :n],
                            xt[:, ci, t * 128:t * 128 + tn],
                            wv[:, ci, off:off + n],
                            start=(ci == 0),
                            stop=(ci == CI - 1),
                        )
                    nc.vector.tensor_copy(vt[:tn, t, off:off + n], ps[:tn, :n])
            return kt, vt

        def scores_softmax(b, kt):
            pcs = []
            for off, n in _chunks(NTP, 512):
                ps = psB.tile([H, 512], F32, tag="sm")
                for ct in range(CI):
                    nc.tensor.matmul(
                        ps[:, :n],
                        qb[:, ct, b, :],
                        kt[:, ct, off:off + n],
                        start=(ct == 0),
                        stop=(ct == CI - 1),
                    )
                pcs.append((ps, off, n))
            st = spool.tile([H, 6], F32, tag="stat")
            for i, (ps, off, n) in enumerate(pcs):
                nr = min(n, NT - off)  # real (non-pad) tokens in this chunk
                nc.vector.reduce_max(st[:, i:i + 1], ps[:, :nr], axis=mybir.AxisListType.X, negate=True)
            nc.vector.tensor_tensor(st[:, 2:3], st[:, 0:1], st[:, 1:2], op=mybir.AluOpType.min)
            p_sb = spool.tile([H, NTP], F32, tag="p")
            nc.vector.memset(p_sb[:, NT:NTP], 0.0)
            for i, (ps, off, n) in enumerate(pcs):
                nr = min(n, NT - off)
                nc.scalar.activation(
                    p_sb[:, off:off + nr],
                    ps[:, :nr],
                    func=mybir.ActivationFunctionType.Exp,
                    bias=st[:, 2:3],
                    scale=1.0,
                    accum_out=st[:, 3 + i:4 + i],
                )
            rs = spool.tile([H, 1], F32, tag="rs")
            nc.vector.tensor_add(st[:, 5:6], st[:, 3:4], st[:, 4:5])
            nc.vector.reciprocal(rs[:], st[:, 5:6])
            return p_sb, rs

        def post_attention(b, p_sb, rs, vt):
            at = spool.tile([128, TT, H], F32R, tag="at")
            for t in range(TT):
                tn = 128 if t < TT - 1 else LAST_T
                pst = psB.tile([128, H], F32, tag="sm")
                nc.tensor.transpose(pst[:tn, :], p_sb[:, t * 128:t * 128 + tn], ident[:H, :H])
                nc.vector.tensor_copy(at[:tn, t, :], pst[:tn, :])
            pv = spool.tile([H, C], F32R, tag="pv")
            for off, n in _chunks(C, 512):
                ps = psB.tile([H, 512], F32, tag="sm")
                for t in range(TT):
                    tn = 128 if t < TT - 1 else LAST_T
                    nc.tensor.matmul(
                        ps[:, :n],
                        at[:tn, t, :],
                        vt[:tn, t, off:off + n],
                        start=(t == 0),
                        stop=(t == TT - 1),
                    )
                # keep only each head's own 64-channel block, scaled by 1/sum
                nc.vector.tensor_tensor(
                    pv[:, off:off + n], ps[:, :n], bm[:, off:off + n],
                    op=mybir.AluOpType.mult,
                )
                nc.vector.tensor_scalar_mul(
                    pv[:, off:off + n], pv[:, off:off + n], rs[:, 0:1])
            for off, n in _chunks(C, 512):
                ps = psB.tile([BL, 512], F32, tag="sm")
                nc.tensor.matmul(
                    ps[:, :n],
                    oh[:, b, :],
                    pv[:, off:off + n],
                    start=True,
                    stop=True,
                )
                # row b of ps holds attn_out for element b; other rows are 0
                nc.vector.tensor_add(
                    attn_out[:, off:off + n], attn_out[:, off:off + n], ps[:, :n])

        prev = None
        for b in range(BL):
            kt, vt = kv_gemm(b)
            if prev is not None:
                post_attention(*prev)
            p_sb, rs = scores_softmax(b, kt)
            prev = (b, p_sb, rs, vt)
        post_attention(*prev)

        # ---- epilogue: y = (x_cls_p + attn_out) @ W_back + b_back
        orT = const.tile([128, CI, BL], F32R)
        for ct in range(CI):
            pst = psB.tile([128, BL], F32, tag="sm")
            nc.tensor.transpose(pst[:], attn_out[:, ct * 128:(ct + 1) * 128], ident[:BL, :BL])
            nc.vector.tensor_add(orT[:, ct, :], pst[:], xcp[:, ct, :])
        psy = psB.tile([BL, C_CLS], F32, tag="sm")
        for ci in range(CI):
            nc.tensor.matmul(
                psy[:],
                orT[:, ci, :],
                wb[:, ci, :],
                start=(ci == 0),
                stop=(ci == CI - 1),
            )
        y_sb = spool.tile([BL, C_CLS], F32, tag="y")
        nc.vector.tensor_add(y_sb[:], psy[:], bb[:])
        nc.sync.dma_start(out=dy[:], in_=y_sb[:])
    return nc


_prog = None


def _get_prog():
    global _prog
    if _prog is None:
        nc = bacc.Bacc("TRN2", target_bir_lowering=False)
        _emit(nc)
        nc.compile()
        _prog = nc
    return _prog


def _prep_shared(W_proj, b_proj, W_q, W_kv, W_back, b_back):
    f = np.float32
    wsplit = lambda w, co: np.ascontiguousarray(
        w.reshape(w.shape[0] // 128, 128, co).transpose(1, 0, 2).astype(f))
    shared = {
        "W_proj": wsplit(W_proj, C),
        "b_projT": np.ascontiguousarray(b_proj.reshape(CI, 128).T.astype(f)),
        "W_q": wsplit((W_q * SCALE).astype(f), C),
        "W_k": wsplit(W_kv[:, :C], C),
        "W_v": wsplit(W_kv[:, C:], C),
        "W_back": wsplit(W_back, C_CLS),
        "b_back": np.ascontiguousarray(np.tile(b_back.astype(f)[None, :], (BL, 1))),
        "onehot": np.ascontiguousarray(
            np.tile(np.eye(BL, dtype=f)[None], (H, 1, 1))),
        "blockmask": np.ascontiguousarray(
            np.repeat(np.eye(H, dtype=f), HD, axis=1)),
    }
    return shared


RUN_KWARGS = {}
LAST_RESULT = None


def kernel(x_cls, x_img, W_proj, b_proj, W_q, W_kv, W_back, b_back):
    global LAST_RESULT
    nc = _get_prog()
    x_cls = np.asarray(x_cls, dtype=np.float32)
    x_img = np.asarray(x_img, dtype=np.float32)
    shared = _prep_shared(
        np.asarray(W_proj, np.float32), np.asarray(b_proj, np.float32),
        np.asarray(W_q, np.float32), np.asarray(W_kv, np.float32),
        np.asarray(W_back, np.float32), np.asarray(b_back, np.float32))
    in_maps = []
    for core in range(NCORES):
        sl = slice(core * BL, (core + 1) * BL)
        xi = x_img[sl]  # [BL, 576, 768]
        xT = np.ascontiguousarray(
            xi.transpose(0, 2, 1).reshape(BL, CI, 128, N_IMG).transpose(0, 2, 1, 3))
        xcT = np.ascontiguousarray(
            x_cls[sl, 0, :].T.reshape(CIC, 128, BL).transpose(1, 0, 2))
        m = dict(shared)
        m["xT_img"] = xT
        m["x_clsT"] = xcT
        in_maps.append(m)
    res = run_bass_kernel_spmd(nc, in_maps, core_ids=list(range(NCORES)), **RUN_KWARGS)
    LAST_RESULT = res
    y = np.concatenate([r["y"] for r in res.results], axis=0).reshape(B, 1, C_CLS)
    return y
